# revision 7
# baseline (speedup 1.0000x reference)
"""CrossAlignMatrix kernel for 8x TRN2 NeuronCores.

out = softmax_j(clip(c.w_c + q.w_q + (c*w_cq).q^T + biases, +-15) + logmask) @ q @ W_out.T + b_out

Data-parallel over batch B=16: 2 batches per core. Three O(L^2 D) bf16
matmuls per batch (scores, attn@q, @W_out.T) with fp32 PSUM accumulate.

v2 design notes (from trace analysis of v1):
- Scalar-engine exp reads scores directly from PSUM with the per-j row
  score as the activation *bias* (exp(s + sqb)); the clip becomes a
  post-exp DVE clamp to [e^-15, e^15] (exactly equivalent, exp monotone).
- Softmax denominators: DVE-side jt-tree accumulation of p into one
  [128,512] tile per i-chunk, then a single ones-column matmul per chunk
  (was 16 full PE matmuls per batch).
- Every stationary 128x128 weight tile feeds TWO 512-wide moving streams
  (two PSUM banks) to amortize the exposed LDWEIGHTS cost (~46ns/MM).
  Batch 0's scores phase stays single-stream so compute can start before
  the second i-chunk of cT lands.
- DMA order: tiny sqb tensors first (v1 queued them behind 4MB of bulk
  input on the single FIFO DMA queue, stalling the DVE->ACT chain and
  re-throttling the PE); per-jb granularity for batch 0's qaugT; one
  fat DMA per remaining tensor (DMA issue costs ~0.7us each on the sync
  engine).
- Denominator reciprocal still via DRAM round-trip transpose; issued
  early in phase 2, consumed in phase 3.
"""
import numpy as np
import ml_dtypes

import concourse.bass as bass
import concourse.bacc as bacc
import concourse.mybir as mybir
from concourse.tile import TileContext
from concourse.bass_utils import run_bass_kernel_spmd

f32 = mybir.dt.float32
bf16 = mybir.dt.bfloat16
BF = ml_dtypes.bfloat16

B, LC, LQ, D = 16, 1024, 1024, 1024
NCORES = 8
G = B // NCORES          # batches per core
NT = D // 128            # 8 tiles of 128 along any contracted dim
NCH = 2                  # 512-wide free chunks per 1024
CH = 512
E_LO = float(np.exp(-15.0))
E_HI = float(np.exp(15.0))

_cache = {}


def _build(add_bout: bool, masked: bool):
    nc = bacc.Bacc(None, target_bir_lowering=False)
    AT = mybir.ActivationFunctionType
    OP = mybir.AluOpType

    qaugT = nc.dram_tensor("qaugT", [G, 128, NT, NT, 128], bf16, kind="ExternalInput")
    cT = nc.dram_tensor("cT", [G, 128, NCH, NT, CH], bf16, kind="ExternalInput")
    qnat = nc.dram_tensor("qnat", [G, 128, NT, D], bf16, kind="ExternalInput")
    sqb = nc.dram_tensor("sqb", [G, 128, NT], f32, kind="ExternalInput")
    WT = nc.dram_tensor("WT", [128, NT, D], bf16, kind="ExternalInput")
    mk = (nc.dram_tensor("mk", [G, 128, NT], f32, kind="ExternalInput")
          if masked else None)
    bout = (nc.dram_tensor("bout_rep", [128, D], f32, kind="ExternalInput")
            if add_bout else None)
    out = nc.dram_tensor("out", [G, LC, D], f32, kind="ExternalOutput")

    with TileContext(nc) as tc:
        with (
            tc.tile_pool(name="big", bufs=1) as big,
            tc.tile_pool(name="small", bufs=1) as small,
            tc.tile_pool(name="accp", bufs=4) as accp,
            tc.tile_pool(name="ostg", bufs=3) as ostg,
            tc.tile_pool(name="ps_s", bufs=4, space="PSUM") as ps_s,
            tc.tile_pool(name="ps_mm", bufs=4, space="PSUM") as ps_mm,
            tc.tile_pool(name="dram", bufs=2, space="DRAM") as dram,
        ):
            ones_col = small.tile([128, 1], bf16, tag="ones")
            nc.vector.memset(ones_col, 1.0)
            # PE warmup: junk matmuls un-throttle HAM while input DMAs land
            wu_sb = small.tile([128, 128], bf16, tag="wu")
            nc.vector.memset(wu_sb, 0.0)
            wu_ps = ps_mm.tile([128, 128], f32, tag="mm")
            for _ in range(36):
                nc.tensor.matmul(wu_ps[0:1, :], ones_col, wu_sb, start=True, stop=True)

            # --- all big tiles up front (both batches live simultaneously) ---
            qaugT_sb = [big.tile([128, NT, NT, 128], bf16, tag=f"qaugT{g}", name=f"qaugT_sb{g}") for g in range(G)]
            cT_sb = [big.tile([128, NCH, NT, CH], bf16, tag=f"cT{g}", name=f"cT_sb{g}") for g in range(G)]
            qnat_sb = [big.tile([128, NT, D], bf16, tag=f"qnat{g}", name=f"qnat_sb{g}") for g in range(G)]
            p_sb = [big.tile([128, NT, LC], bf16, tag=f"p{g}", name=f"p_sb{g}") for g in range(G)]
            c2q_sb = [big.tile([128, NT, LC], bf16, tag=f"c2q{g}", name=f"c2q_sb{g}") for g in range(G)]
            WT_sb = big.tile([128, NT, D], bf16, tag="WT")
            sqb_sb = [small.tile([128, NT], f32, tag=f"sqb{g}", name=f"sqb_sb{g}") for g in range(G)]
            mk_sb = ([small.tile([128, NT], f32, tag=f"mk{g}", name=f"mk_sb{g}") for g in range(G)]
                     if masked else None)
            bout_sb = small.tile([128, D], f32, tag="bout") if add_bout else None

            # --- input DMAs ---
            # Tiny per-partition tensors go on the scalar engine's HWDGE
            # ring so they never queue behind bulk input on the sync ring.
            for g in range(G):
                nc.scalar.dma_start(out=sqb_sb[g], in_=sqb[g])
                if masked:
                    nc.scalar.dma_start(out=mk_sb[g], in_=mk[g])
            # Bulk inputs on the sync ring, strictly in first-use order;
            # batch 0's first score group is fed by jb0 + two cT half-chunks
            # so matmuls start as early as the FIFO queue can deliver.
            nc.sync.dma_start(out=qaugT_sb[0][:, 0], in_=qaugT[0, :, 0])
            nc.sync.dma_start(out=cT_sb[0][:, 0, 0:4], in_=cT[0, :, 0, 0:4])
            nc.sync.dma_start(out=cT_sb[0][:, 0, 4:8], in_=cT[0, :, 0, 4:8])
            for jb in range(1, NT):
                nc.sync.dma_start(out=qaugT_sb[0][:, jb], in_=qaugT[0, :, jb])
            nc.sync.dma_start(out=cT_sb[0][:, 1], in_=cT[0, :, 1])
            nc.sync.dma_start(out=qnat_sb[0], in_=qnat[0])
            nc.sync.dma_start(out=qaugT_sb[1], in_=qaugT[1])
            nc.sync.dma_start(out=cT_sb[1], in_=cT[1])
            nc.sync.dma_start(out=WT_sb, in_=WT[:, :, :])
            nc.sync.dma_start(out=qnat_sb[1], in_=qnat[1])
            if add_bout:
                nc.sync.dma_start(out=bout_sb, in_=bout[:, :])

            def exp_clamp(g, jb, n, s_ps):
                isl = slice(n * CH, (n + 1) * CH)
                dst = p_sb[g][:, jb, isl]
                nc.scalar.activation(out=dst, in_=s_ps, func=AT.Exp,
                                     bias=sqb_sb[g][:, jb:jb + 1], scale=1.0)
                nc.vector.tensor_scalar(out=dst, in0=dst,
                                        scalar1=E_LO, scalar2=E_HI,
                                        op0=OP.max, op1=OP.min)
                if masked:
                    nc.vector.tensor_scalar(out=dst, in0=dst,
                                            scalar1=mk_sb[g][:, jb:jb + 1],
                                            scalar2=None, op0=OP.mult)

            def den_acc(g, n):
                isl = slice(n * CH, (n + 1) * CH)
                acc = accp.tile([128, CH], bf16, tag="acc")
                nc.vector.tensor_add(acc, p_sb[g][:, 0, isl], p_sb[g][:, 1, isl])
                for jt in range(2, NT):
                    nc.vector.tensor_add(acc, acc, p_sb[g][:, jt, isl])
                return acc

            for g in range(G):
                # ---- phase 1: scores -> p ----
                accs = [None, None]
                if g == 0:
                    # single stream: start before cT chunk 1 lands
                    for n in range(NCH):
                        for jb in range(NT):
                            s_ps = ps_s.tile([128, CH], f32, tag="s")
                            for dt in range(NT):
                                nc.tensor.matmul(
                                    s_ps, qaugT_sb[g][:, jb, dt, :],
                                    cT_sb[g][:, n, dt, :],
                                    start=(dt == 0), stop=(dt == NT - 1))
                            exp_clamp(g, jb, n, s_ps)
                        accs[n] = den_acc(g, n)
                else:
                    # paired streams: one weight load per two matmuls
                    for jb in range(NT):
                        s0 = ps_s.tile([128, CH], f32, tag="s")
                        s1 = ps_s.tile([128, CH], f32, tag="s")
                        for dt in range(NT):
                            w = qaugT_sb[g][:, jb, dt, :]
                            nc.tensor.matmul(s0, w, cT_sb[g][:, 0, dt, :],
                                             start=(dt == 0), stop=(dt == NT - 1))
                            nc.tensor.matmul(s1, w, cT_sb[g][:, 1, dt, :],
                                             start=(dt == 0), stop=(dt == NT - 1))
                        exp_clamp(g, jb, 0, s0)
                        exp_clamp(g, jb, 1, s1)
                    accs[0] = den_acc(g, 0)
                    accs[1] = den_acc(g, 1)

                den_row = small.tile([1, LC], f32, tag="den_row")
                rcp = small.tile([128, NT], f32, tag="rcp")

                # ---- phase 2: c2qT[d, i] = sum_j qnat[j, d] * p[j, i] ----
                for m in range(NT):
                    c0 = ps_mm.tile([128, CH], f32, tag="mm")
                    c1 = ps_mm.tile([128, CH], f32, tag="mm")
                    for jt in range(NT):
                        w = qnat_sb[g][:, jt, m * 128:(m + 1) * 128]
                        nc.tensor.matmul(c0, w, p_sb[g][:, jt, 0:CH],
                                         start=(jt == 0), stop=(jt == NT - 1))
                        nc.tensor.matmul(c1, w, p_sb[g][:, jt, CH:2 * CH],
                                         start=(jt == 0), stop=(jt == NT - 1))
                    nc.vector.tensor_copy(out=c2q_sb[g][:, m, 0:CH], in_=c0)
                    nc.scalar.copy(out=c2q_sb[g][:, m, CH:2 * CH], in_=c1)

                    if m == 0:
                        # denominators: one ones-matmul per i-chunk, then
                        # DRAM round-trip row->columns transpose + reciprocal
                        for n in range(NCH):
                            isl = slice(n * CH, (n + 1) * CH)
                            den_ps = ps_s.tile([1, CH], f32, tag="s")
                            nc.tensor.matmul(den_ps, ones_col, accs[n],
                                             start=True, stop=True)
                            nc.scalar.copy(out=den_row[0:1, isl], in_=den_ps)
                        den_dram = dram.tile([1, LC], f32, tag="dend")
                        nc.sync.dma_start(out=den_dram, in_=den_row)
                        den_cols = small.tile([128, NT], f32, tag="den_cols")
                        nc.sync.dma_start(
                            out=den_cols,
                            in_=den_dram.rearrange("a (t p) -> p (t a)", p=128))
                        nc.vector.reciprocal(out=rcp, in_=den_cols)

                # ---- phase 3: out[i, e] = (c2qT.T @ WT) * rcp[i] (+ b_out) ----
                for ib in range(NT):
                    o0 = ps_mm.tile([128, CH], f32, tag="mm")
                    o1 = ps_mm.tile([128, CH], f32, tag="mm")
                    for dt in range(NT):
                        w = c2q_sb[g][:, dt, ib * 128:(ib + 1) * 128]
                        nc.tensor.matmul(o0, w, WT_sb[:, dt, 0:CH],
                                         start=(dt == 0), stop=(dt == NT - 1))
                        nc.tensor.matmul(o1, w, WT_sb[:, dt, CH:2 * CH],
                                         start=(dt == 0), stop=(dt == NT - 1))
                    rsc = rcp[:, ib:ib + 1]
                    # output DMAs issue from the scalar engine's ring so the
                    # final tiles never wait behind sync-ring bulk, and the
                    # two issues per group don't serialize with inputs
                    o_sb0 = ostg.tile([128, CH], f32, tag="o")
                    nc.scalar.activation(out=o_sb0, in_=o0, func=AT.Copy, scale=rsc)
                    if add_bout:
                        nc.vector.tensor_add(o_sb0, o_sb0, bout_sb[:, 0:CH])
                    nc.scalar.dma_start(
                        out=out[g, ib * 128:(ib + 1) * 128, 0:CH], in_=o_sb0)
                    o_sb1 = ostg.tile([128, CH], f32, tag="o")
                    nc.vector.tensor_scalar(out=o_sb1, in0=o1, scalar1=rsc,
                                            scalar2=None, op0=OP.mult)
                    if add_bout:
                        nc.vector.tensor_add(o_sb1, o_sb1, bout_sb[:, CH:2 * CH])
                    nc.sync.dma_start(
                        out=out[g, ib * 128:(ib + 1) * 128, CH:2 * CH], in_=o_sb1)

    nc.compile()
    return nc


def kernel(c, q, q_mask, w_c, b_c, w_q, b_q, w_cq, b_cq, W_out, b_out):
    c = np.asarray(c, dtype=np.float32)
    q = np.asarray(q, dtype=np.float32)
    q_mask = np.asarray(q_mask)
    w_c = np.asarray(w_c, dtype=np.float32)
    w_q = np.asarray(w_q, dtype=np.float32)
    w_cq = np.asarray(w_cq, dtype=np.float32)
    W_out = np.asarray(W_out, dtype=np.float32)
    b_sum = float(b_c) + float(b_q) + float(b_cq)
    b_out = np.asarray(b_out, dtype=np.float32)
    add_bout = bool(np.any(b_out != 0.0))
    masked = not bool(np.all(q_mask == 1))

    key = (add_bout, masked)
    if key not in _cache:
        _cache[key] = _build(add_bout, masked)
    nc = _cache[key]

    # host layout prep (O(N^2) data movement only)
    qaug = q * w_cq + w_c
    qaugT = np.ascontiguousarray(
        qaug.reshape(B, NT, 128, NT, 128).transpose(0, 4, 1, 3, 2)).astype(BF)
    cT = np.ascontiguousarray(
        c.reshape(B, NCH, CH, NT, 128).transpose(0, 4, 1, 3, 2)).astype(BF)
    qnat = np.ascontiguousarray(
        q.reshape(B, NT, 128, D).transpose(0, 2, 1, 3)).astype(BF)
    sq = q.astype(np.float32) @ w_q + b_sum                     # [B, LQ]
    sqb = np.ascontiguousarray(sq.reshape(B, NT, 128).transpose(0, 2, 1))
    WTf = np.ascontiguousarray(
        W_out.T.reshape(NT, 128, D).transpose(1, 0, 2)).astype(BF)

    in_maps = []
    for core in range(NCORES):
        gs = slice(core * G, (core + 1) * G)
        m = {
            "qaugT": qaugT[gs], "cT": cT[gs], "qnat": qnat[gs],
            "sqb": sqb[gs], "WT": WTf,
        }
        if masked:
            mkf = np.ascontiguousarray(
                (q_mask != 0).astype(np.float32).reshape(B, NT, 128)
                .transpose(0, 2, 1))
            m["mk"] = mkf[gs]
        if add_bout:
            m["bout_rep"] = np.broadcast_to(b_out, (128, D)).copy()
        in_maps.append(m)

    res = run_bass_kernel_spmd(nc, in_maps, list(range(NCORES)))
    kernel._last_res = res

    out = np.empty((B, LC, D), dtype=np.float32)
    for core in range(NCORES):
        out[core * G:(core + 1) * G] = res.results[core]["out"]
    return out


# revision 9
# speedup vs baseline: 1.0003x; 1.0003x over previous
"""CrossAlignMatrix kernel for 8x TRN2 NeuronCores.

out = softmax_j(clip(c.w_c + q.w_q + (c*w_cq).q^T + biases, +-15) + logmask) @ q @ W_out.T + b_out

Data-parallel over batch B=16: 2 batches per core. Three O(L^2 D) bf16
matmuls per batch (scores, attn@q, @W_out.T) with fp32 PSUM accumulate.

v2 design notes (from trace analysis of v1):
- Scalar-engine exp reads scores directly from PSUM with the per-j row
  score as the activation *bias* (exp(s + sqb)); the clip becomes a
  post-exp DVE clamp to [e^-15, e^15] (exactly equivalent, exp monotone).
- Softmax denominators: DVE-side jt-tree accumulation of p into one
  [128,512] tile per i-chunk, then a single ones-column matmul per chunk
  (was 16 full PE matmuls per batch).
- Every stationary 128x128 weight tile feeds TWO 512-wide moving streams
  (two PSUM banks) to amortize the exposed LDWEIGHTS cost (~46ns/MM).
  Batch 0's scores phase stays single-stream so compute can start before
  the second i-chunk of cT lands.
- DMA order: tiny sqb tensors first (v1 queued them behind 4MB of bulk
  input on the single FIFO DMA queue, stalling the DVE->ACT chain and
  re-throttling the PE); per-jb granularity for batch 0's qaugT; one
  fat DMA per remaining tensor (DMA issue costs ~0.7us each on the sync
  engine).
- Denominator reciprocal still via DRAM round-trip transpose; issued
  early in phase 2, consumed in phase 3.
"""
import numpy as np
import ml_dtypes

import concourse.bass as bass
import concourse.bacc as bacc
import concourse.mybir as mybir
from concourse.tile import TileContext
from concourse.bass_utils import run_bass_kernel_spmd

f32 = mybir.dt.float32
bf16 = mybir.dt.bfloat16
BF = ml_dtypes.bfloat16

B, LC, LQ, D = 16, 1024, 1024, 1024
NCORES = 8
G = B // NCORES          # batches per core
NT = D // 128            # 8 tiles of 128 along any contracted dim
NCH = 2                  # 512-wide free chunks per 1024
CH = 512
E_LO = float(np.exp(-15.0))
E_HI = float(np.exp(15.0))

_cache = {}


def _build(add_bout: bool, masked: bool):
    nc = bacc.Bacc(None, target_bir_lowering=False)
    AT = mybir.ActivationFunctionType
    OP = mybir.AluOpType

    qaugT = nc.dram_tensor("qaugT", [G, 128, NT, NT, 128], bf16, kind="ExternalInput")
    cT = nc.dram_tensor("cT", [G, 128, NCH, NT, CH], bf16, kind="ExternalInput")
    qnat = nc.dram_tensor("qnat", [G, 128, NT, D], bf16, kind="ExternalInput")
    sqb = nc.dram_tensor("sqb", [G, 128, NT], f32, kind="ExternalInput")
    WT = nc.dram_tensor("WT", [128, NT, D], bf16, kind="ExternalInput")
    mk = (nc.dram_tensor("mk", [G, 128, NT], f32, kind="ExternalInput")
          if masked else None)
    bout = (nc.dram_tensor("bout_rep", [128, D], f32, kind="ExternalInput")
            if add_bout else None)
    out = nc.dram_tensor("out", [G, LC, D], f32, kind="ExternalOutput")

    with TileContext(nc) as tc:
        with (
            tc.tile_pool(name="big", bufs=1) as big,
            tc.tile_pool(name="small", bufs=1) as small,
            tc.tile_pool(name="accp", bufs=4) as accp,
            tc.tile_pool(name="ostg", bufs=3) as ostg,
            tc.tile_pool(name="ps_s", bufs=4, space="PSUM") as ps_s,
            tc.tile_pool(name="ps_mm", bufs=4, space="PSUM") as ps_mm,
            tc.tile_pool(name="dram", bufs=2, space="DRAM") as dram,
        ):
            ones_col = small.tile([128, 1], bf16, tag="ones")
            nc.vector.memset(ones_col, 1.0)
            # PE warmup: junk matmuls un-throttle HAM while input DMAs land
            wu_sb = small.tile([128, 128], bf16, tag="wu")
            nc.vector.memset(wu_sb, 0.0)
            wu_ps = ps_mm.tile([128, 128], f32, tag="mm")
            for _ in range(36):
                nc.tensor.matmul(wu_ps[0:1, :], ones_col, wu_sb, start=True, stop=True)

            # --- all big tiles up front (both batches live simultaneously) ---
            qaugT_sb = [big.tile([128, NT, NT, 128], bf16, tag=f"qaugT{g}", name=f"qaugT_sb{g}") for g in range(G)]
            cT_sb = [big.tile([128, NCH, NT, CH], bf16, tag=f"cT{g}", name=f"cT_sb{g}") for g in range(G)]
            qnat_sb = [big.tile([128, NT, D], bf16, tag=f"qnat{g}", name=f"qnat_sb{g}") for g in range(G)]
            p_sb = [big.tile([128, NT, LC], bf16, tag=f"p{g}", name=f"p_sb{g}") for g in range(G)]
            c2q_sb = [big.tile([128, NT, LC], bf16, tag=f"c2q{g}", name=f"c2q_sb{g}") for g in range(G)]
            WT_sb = big.tile([128, NT, D], bf16, tag="WT")
            sqb_sb = [small.tile([128, NT], f32, tag=f"sqb{g}", name=f"sqb_sb{g}") for g in range(G)]
            mk_sb = ([small.tile([128, NT], f32, tag=f"mk{g}", name=f"mk_sb{g}") for g in range(G)]
                     if masked else None)
            bout_sb = small.tile([128, D], f32, tag="bout") if add_bout else None

            # --- input DMAs ---
            # Tiny per-partition tensors go on the scalar engine's HWDGE
            # ring so they never queue behind bulk input on the sync ring.
            for g in range(G):
                nc.scalar.dma_start(out=sqb_sb[g], in_=sqb[g])
                if masked:
                    nc.scalar.dma_start(out=mk_sb[g], in_=mk[g])
            # Bulk inputs on the sync ring, strictly in first-use order;
            # batch 0's first score group is fed by jb0 + two cT half-chunks
            # so matmuls start as early as the FIFO queue can deliver.
            nc.sync.dma_start(out=qaugT_sb[0][:, 0], in_=qaugT[0, :, 0])
            for dp in range(4):
                nc.sync.dma_start(out=cT_sb[0][:, 0, 2 * dp:2 * dp + 2],
                                  in_=cT[0, :, 0, 2 * dp:2 * dp + 2])
            for jb in range(1, NT):
                nc.sync.dma_start(out=qaugT_sb[0][:, jb], in_=qaugT[0, :, jb])
            nc.sync.dma_start(out=cT_sb[0][:, 1], in_=cT[0, :, 1])
            nc.sync.dma_start(out=qnat_sb[0], in_=qnat[0])
            nc.sync.dma_start(out=qaugT_sb[1], in_=qaugT[1])
            nc.sync.dma_start(out=cT_sb[1], in_=cT[1])
            nc.sync.dma_start(out=WT_sb, in_=WT[:, :, :])
            nc.sync.dma_start(out=qnat_sb[1], in_=qnat[1])
            if add_bout:
                nc.sync.dma_start(out=bout_sb, in_=bout[:, :])

            def exp_clamp(g, jb, n, s_ps):
                isl = slice(n * CH, (n + 1) * CH)
                dst = p_sb[g][:, jb, isl]
                nc.scalar.activation(out=dst, in_=s_ps, func=AT.Exp,
                                     bias=sqb_sb[g][:, jb:jb + 1], scale=1.0)
                nc.vector.tensor_scalar(out=dst, in0=dst,
                                        scalar1=E_LO, scalar2=E_HI,
                                        op0=OP.max, op1=OP.min)
                if masked:
                    nc.vector.tensor_scalar(out=dst, in0=dst,
                                            scalar1=mk_sb[g][:, jb:jb + 1],
                                            scalar2=None, op0=OP.mult)

            def den_acc(g, n):
                isl = slice(n * CH, (n + 1) * CH)
                acc = accp.tile([128, CH], bf16, tag="acc")
                nc.vector.tensor_add(acc, p_sb[g][:, 0, isl], p_sb[g][:, 1, isl])
                for jt in range(2, NT):
                    nc.vector.tensor_add(acc, acc, p_sb[g][:, jt, isl])
                return acc

            for g in range(G):
                # ---- phase 1: scores -> p ----
                accs = [None, None]
                if g == 0:
                    # single stream: start before cT chunk 1 lands
                    for n in range(NCH):
                        for jb in range(NT):
                            s_ps = ps_s.tile([128, CH], f32, tag="s")
                            for dt in range(NT):
                                nc.tensor.matmul(
                                    s_ps, qaugT_sb[g][:, jb, dt, :],
                                    cT_sb[g][:, n, dt, :],
                                    start=(dt == 0), stop=(dt == NT - 1))
                            exp_clamp(g, jb, n, s_ps)
                        accs[n] = den_acc(g, n)
                else:
                    # paired streams: one weight load per two matmuls
                    for jb in range(NT):
                        s0 = ps_s.tile([128, CH], f32, tag="s")
                        s1 = ps_s.tile([128, CH], f32, tag="s")
                        for dt in range(NT):
                            w = qaugT_sb[g][:, jb, dt, :]
                            nc.tensor.matmul(s0, w, cT_sb[g][:, 0, dt, :],
                                             start=(dt == 0), stop=(dt == NT - 1))
                            nc.tensor.matmul(s1, w, cT_sb[g][:, 1, dt, :],
                                             start=(dt == 0), stop=(dt == NT - 1))
                        exp_clamp(g, jb, 0, s0)
                        exp_clamp(g, jb, 1, s1)
                    accs[0] = den_acc(g, 0)
                    accs[1] = den_acc(g, 1)

                den_row = small.tile([1, LC], f32, tag="den_row")
                rcp = small.tile([128, NT], f32, tag="rcp")

                # ---- phase 2: c2qT[d, i] = sum_j qnat[j, d] * p[j, i] ----
                for m in range(NT):
                    c0 = ps_mm.tile([128, CH], f32, tag="mm")
                    c1 = ps_mm.tile([128, CH], f32, tag="mm")
                    for jt in range(NT):
                        w = qnat_sb[g][:, jt, m * 128:(m + 1) * 128]
                        nc.tensor.matmul(c0, w, p_sb[g][:, jt, 0:CH],
                                         start=(jt == 0), stop=(jt == NT - 1))
                        nc.tensor.matmul(c1, w, p_sb[g][:, jt, CH:2 * CH],
                                         start=(jt == 0), stop=(jt == NT - 1))
                    nc.vector.tensor_copy(out=c2q_sb[g][:, m, 0:CH], in_=c0)
                    nc.scalar.copy(out=c2q_sb[g][:, m, CH:2 * CH], in_=c1)

                    if m == 0:
                        # denominators: one ones-matmul per i-chunk, then
                        # DRAM round-trip row->columns transpose + reciprocal
                        for n in range(NCH):
                            isl = slice(n * CH, (n + 1) * CH)
                            den_ps = ps_s.tile([1, CH], f32, tag="s")
                            nc.tensor.matmul(den_ps, ones_col, accs[n],
                                             start=True, stop=True)
                            nc.scalar.copy(out=den_row[0:1, isl], in_=den_ps)
                        den_dram = dram.tile([1, LC], f32, tag="dend")
                        nc.sync.dma_start(out=den_dram, in_=den_row)
                        den_cols = small.tile([128, NT], f32, tag="den_cols")
                        nc.sync.dma_start(
                            out=den_cols,
                            in_=den_dram.rearrange("a (t p) -> p (t a)", p=128))
                        nc.vector.reciprocal(out=rcp, in_=den_cols)

                # ---- phase 3: out[i, e] = (c2qT.T @ WT) * rcp[i] (+ b_out) ----
                for ib in range(NT):
                    o0 = ps_mm.tile([128, CH], f32, tag="mm")
                    o1 = ps_mm.tile([128, CH], f32, tag="mm")
                    for dt in range(NT):
                        w = c2q_sb[g][:, dt, ib * 128:(ib + 1) * 128]
                        nc.tensor.matmul(o0, w, WT_sb[:, dt, 0:CH],
                                         start=(dt == 0), stop=(dt == NT - 1))
                        nc.tensor.matmul(o1, w, WT_sb[:, dt, CH:2 * CH],
                                         start=(dt == 0), stop=(dt == NT - 1))
                    rsc = rcp[:, ib:ib + 1]
                    # output DMAs issue from the scalar engine's ring so the
                    # final tiles never wait behind sync-ring bulk, and the
                    # two issues per group don't serialize with inputs
                    o_sb0 = ostg.tile([128, CH], f32, tag="o")
                    nc.scalar.activation(out=o_sb0, in_=o0, func=AT.Copy, scale=rsc)
                    if add_bout:
                        nc.vector.tensor_add(o_sb0, o_sb0, bout_sb[:, 0:CH])
                    nc.scalar.dma_start(
                        out=out[g, ib * 128:(ib + 1) * 128, 0:CH], in_=o_sb0)
                    o_sb1 = ostg.tile([128, CH], f32, tag="o")
                    nc.vector.tensor_scalar(out=o_sb1, in0=o1, scalar1=rsc,
                                            scalar2=None, op0=OP.mult)
                    if add_bout:
                        nc.vector.tensor_add(o_sb1, o_sb1, bout_sb[:, CH:2 * CH])
                    nc.scalar.dma_start(
                        out=out[g, ib * 128:(ib + 1) * 128, CH:2 * CH], in_=o_sb1)

    nc.compile()
    return nc


def kernel(c, q, q_mask, w_c, b_c, w_q, b_q, w_cq, b_cq, W_out, b_out):
    c = np.asarray(c, dtype=np.float32)
    q = np.asarray(q, dtype=np.float32)
    q_mask = np.asarray(q_mask)
    w_c = np.asarray(w_c, dtype=np.float32)
    w_q = np.asarray(w_q, dtype=np.float32)
    w_cq = np.asarray(w_cq, dtype=np.float32)
    W_out = np.asarray(W_out, dtype=np.float32)
    b_sum = float(b_c) + float(b_q) + float(b_cq)
    b_out = np.asarray(b_out, dtype=np.float32)
    add_bout = bool(np.any(b_out != 0.0))
    masked = not bool(np.all(q_mask == 1))

    key = (add_bout, masked)
    if key not in _cache:
        _cache[key] = _build(add_bout, masked)
    nc = _cache[key]

    # host layout prep (O(N^2) data movement only)
    qaug = q * w_cq + w_c
    qaugT = np.ascontiguousarray(
        qaug.reshape(B, NT, 128, NT, 128).transpose(0, 4, 1, 3, 2)).astype(BF)
    cT = np.ascontiguousarray(
        c.reshape(B, NCH, CH, NT, 128).transpose(0, 4, 1, 3, 2)).astype(BF)
    qnat = np.ascontiguousarray(
        q.reshape(B, NT, 128, D).transpose(0, 2, 1, 3)).astype(BF)
    sq = q.astype(np.float32) @ w_q + b_sum                     # [B, LQ]
    sqb = np.ascontiguousarray(sq.reshape(B, NT, 128).transpose(0, 2, 1))
    WTf = np.ascontiguousarray(
        W_out.T.reshape(NT, 128, D).transpose(1, 0, 2)).astype(BF)

    in_maps = []
    for core in range(NCORES):
        gs = slice(core * G, (core + 1) * G)
        m = {
            "qaugT": qaugT[gs], "cT": cT[gs], "qnat": qnat[gs],
            "sqb": sqb[gs], "WT": WTf,
        }
        if masked:
            mkf = np.ascontiguousarray(
                (q_mask != 0).astype(np.float32).reshape(B, NT, 128)
                .transpose(0, 2, 1))
            m["mk"] = mkf[gs]
        if add_bout:
            m["bout_rep"] = np.broadcast_to(b_out, (128, D)).copy()
        in_maps.append(m)

    res = run_bass_kernel_spmd(nc, in_maps, list(range(NCORES)))
    kernel._last_res = res

    out = np.empty((B, LC, D), dtype=np.float32)
    for core in range(NCORES):
        out[core * G:(core + 1) * G] = res.results[core]["out"]
    return out
